# revision 1
# baseline (speedup 1.0000x reference)
"""Trainium2 Bass kernel for nn_DetailCapture (deformable-conv detail-capture block).

Sharding: 8 cores = batch (2) x row-blocks (4 x 32 rows). Each core computes its
[b, :, y0:y0+32, :] output slice from host-staged per-core input slabs (halos are
baked into the slabs, so no collectives are needed).

Per-core device pipeline (per 32-row block):
  1. Offset convs for all 3 branches via PE matmuls (9 taps x 2 k-tiles, PSUM acc)
  2. PE-transpose offsets to [pixel, 54] layout
  3. Floor/frac/bilinear-weight/index tables on DVE (is_ge floor chains)
  4. Per row x branch: indirect-DMA gather of the 4 bilinear taps (2 contiguous
     x-taps per index; y-taps via idx and idx+W) from a transposed bf16 image slab
  5. Bilinear apply: scalar_tensor_tensor chains (per-pixel weights as per-partition
     scalars), depthwise merge with def_w, LayerNorm2d + gelu (sigmoid form)
  6. 1x1 conv (PE), branch sum, LN, per-pixel MLP (PE matmuls), residual, LN
"""
import sys
import numpy as np

sys.path.insert(0, "/opt/trn_rl_repo")

import concourse.bass as bass
import concourse.bacc as bacc
import concourse.mybir as mybir
import concourse.tile as tile
from concourse.bass import AP

P = 128
B, C, H, W = 2, 256, 128, 128
RB = 32                # rows per core block
NCORES = 8
DILS = (1, 9, 12)
NBR = 3
K = 9
HALO = 12              # conv halo (max dil)
SROWS = RB + 2 * HALO  # 56 rows in conv slab
WP = W + 2 * HALO      # 152 padded width
TPAD = 16              # xT slab vertical pad rows (each side)
TROWS = (RB + 2 * TPAD) * W
TR_ALLOC = TROWS + 2 * W
A_OP = mybir.AluOpType
F32 = mybir.dt.float32
BF16 = mybir.dt.bfloat16
I32 = mybir.dt.int32
AF = mybir.ActivationFunctionType

# gelu(x) ~= x * sigmoid(G_A*x + G_B*x^3)  (tanh-form gelu)
G_A = 1.5957691216057308
G_B = 0.07135481627248
EPS = 1e-6

_COMPILED = None
DEBUG_TAPS = False


def build_program():
    nc = bacc.Bacc(None, target_bir_lowering=False, debug=False,
                   num_swdge_queues=4)

    # ---------------- DRAM I/O (host-staged layouts) ----------------
    d_xslab = nc.dram_tensor("xslab", [2, P, SROWS * WP], BF16, kind="ExternalInput")
    d_xT = nc.dram_tensor("xT", [TR_ALLOC, 2 * C], BF16, kind="ExternalInput")
    d_offw = nc.dram_tensor("offw", [P, NBR * K * 2 * 18], BF16, kind="ExternalInput")
    d_convw = nc.dram_tensor("convw", [P, 4 * P], BF16, kind="ExternalInput")
    d_convb = nc.dram_tensor("convb", [P, 2], F32, kind="ExternalInput")
    d_defw = nc.dram_tensor("defw", [P, NBR * K * C], BF16, kind="ExternalInput")
    d_w1T = nc.dram_tensor("w1T", [P, 2 * 512], F32, kind="ExternalInput")
    d_b1row = nc.dram_tensor("b1row", [1, 512], F32, kind="ExternalInput")
    d_w2T = nc.dram_tensor("w2T", [P, 4 * C], F32, kind="ExternalInput")
    d_b2row = nc.dram_tensor("b2row", [1, C], F32, kind="ExternalInput")
    d_ybrel = nc.dram_tensor("ybrel", [P, NBR * RB * K], BF16, kind="ExternalInput")
    d_ybabs = nc.dram_tensor("ybabs", [P, NBR * RB * K], BF16, kind="ExternalInput")
    d_xvb = nc.dram_tensor("xvb", [P, NBR * RB * K], BF16, kind="ExternalInput")
    d_ident = nc.dram_tensor("identf", [P, P], F32, kind="ExternalInput")
    d_identb = nc.dram_tensor("identb", [P, P], BF16, kind="ExternalInput")
    d_ones1 = nc.dram_tensor("ones1", [1, P], F32, kind="ExternalInput")
    d_out = nc.dram_tensor("out", [RB * P, C], F32, kind="ExternalOutput")
    if DEBUG_TAPS:
        d_dbg_offT = nc.dram_tensor("dbg_offT", [P, RB * 96], F32, kind="ExternalOutput")
        d_dbg_s = nc.dram_tensor("dbg_s", [P, 4 * NBR * RB * K], F32, kind="ExternalOutput")
        d_dbg_idx = nc.dram_tensor("dbg_idx", [P, 2 * NBR * RB * K], I32, kind="ExternalOutput")
        d_dbg_g = nc.dram_tensor("dbg_g", [P, 2 * K * 2 * C], F32, kind="ExternalOutput")
        d_dbg_acc = nc.dram_tensor("dbg_acc", [P, C], F32, kind="ExternalOutput")
        d_dbg_glu = nc.dram_tensor("dbg_glu", [P, NBR * C], F32, kind="ExternalOutput")
        d_dbg_tot = nc.dram_tensor("dbg_tot", [P, C], F32, kind="ExternalOutput")
        d_dbg_h = nc.dram_tensor("dbg_h", [P, 512], F32, kind="ExternalOutput")

    with tile.TileContext(nc) as tc:
        import contextlib
        ctx = contextlib.ExitStack()
        with ctx:
            cpool = ctx.enter_context(tc.tile_pool(name="const", bufs=1))
            spool = ctx.enter_context(tc.tile_pool(name="slab", bufs=1))
            tpool = ctx.enter_context(tc.tile_pool(name="tables", bufs=1))
            wpool = ctx.enter_context(tc.tile_pool(name="work", bufs=1))
            gpool = ctx.enter_context(tc.tile_pool(name="gath", bufs=2))
            apool = ctx.enter_context(tc.tile_pool(name="apply", bufs=2))
            mpool = ctx.enter_context(tc.tile_pool(name="mlp", bufs=2))
            pspool = ctx.enter_context(tc.tile_pool(name="ps", bufs=2, space="PSUM"))

            # ---- constants ----
            def load_const(name, dram, shape, dtype):
                t = cpool.tile(shape, dtype, tag=name, name=name)
                nc.sync.dma_start(t[:], dram[:])
                return t

            ident = load_const("ident", d_ident, [P, P], F32)
            identb = load_const("identb", d_identb, [P, P], BF16)
            ones1 = load_const("ones1", d_ones1, [1, P], F32)
            convb = load_const("convb", d_convb, [P, 2], F32)
            offw = load_const("offw", d_offw, [P, NBR * K * 2 * 18], BF16)
            convw = load_const("convw", d_convw, [P, 4 * P], BF16)
            defw = load_const("defw", d_defw, [P, NBR * K * C], BF16)
            w1T = load_const("w1T", d_w1T, [P, 2 * 512], F32)
            w2T = load_const("w2T", d_w2T, [P, 4 * C], F32)
            b1row = load_const("b1row", d_b1row, [1, 512], F32)
            b2row = load_const("b2row", d_b2row, [1, C], F32)
            ybrel = load_const("ybrel", d_ybrel, [P, NBR * RB * K], BF16)
            ybabs = load_const("ybabs", d_ybabs, [P, NBR * RB * K], BF16)
            xvb = load_const("xvb", d_xvb, [P, NBR * RB * K], BF16)

            xslab = [spool.tile([P, SROWS, WP], BF16, tag=f"xs{kt}", name=f"xs{kt}") for kt in range(2)]
            for kt in range(2):
                nc.sync.dma_start(
                    xslab[kt][:], d_xslab[kt].rearrange("p (r w) -> p r w", w=WP))

            def offw_ap(br, tap, kt):
                base = ((br * K + tap) * 2 + kt) * 18
                return offw[:, base:base + 18]

            # ---- stage 1+2: offset convs -> transpose -> offT [128, RB, 96] ----
            # (each branch's 18 offset channels at partition base 32*br: PE
            #  output base partitions must be 32-aligned)
            offT = tpool.tile([P, RB, 96], F32)
            nc.vector.memset(offT[:], 0.0)
            for chunk in range(RB // 4):          # 4 rows = 512 px per chunk
                ps_off = pspool.tile([96, 512], F32, tag="ps2k", bufs=2)
                for br in range(NBR):
                    dil = DILS[br]
                    for tap in range(K):
                        dy, dx = tap // 3, tap % 3
                        srow = 4 * chunk + HALO + (dy - 1) * dil
                        scol = HALO + (dx - 1) * dil
                        for kt in range(2):
                            nc.tensor.matmul(
                                ps_off[br * 32:br * 32 + 18, :],
                                lhsT=offw_ap(br, tap, kt),
                                rhs=xslab[kt][:, srow:srow + 4, scol:scol + P],
                                start=(tap == 0 and kt == 0),
                                stop=(tap == K - 1 and kt == 1),
                            )
                osb = wpool.tile([96, 512], F32, tag="osb")
                for br in range(NBR):
                    nc.any.tensor_copy(osb[br * 32:br * 32 + 18, :],
                                       ps_off[br * 32:br * 32 + 18, :])
                for sub in range(4):
                    r = chunk * 4 + sub
                    for br in range(NBR):
                        ps_t = pspool.tile([P, 18], F32, tag="ps_sm", bufs=1)
                        nc.tensor.transpose(
                            ps_t[:], osb[br * 32:br * 32 + 18, sub * P:(sub + 1) * P],
                            ident[br * 32:br * 32 + 18, br * 32:br * 32 + 18])
                        nc.any.tensor_copy(offT[:, r, br * 32:br * 32 + 18], ps_t[:])

            # ---- stage 3: per-branch tables ----
            s00 = tpool.tile([P, NBR, RB, K], F32)
            s01 = tpool.tile([P, NBR, RB, K], F32)
            s10 = tpool.tile([P, NBR, RB, K], F32)
            s11 = tpool.tile([P, NBR, RB, K], F32)
            idx0 = tpool.tile([P, NBR, RB, K], I32)

            shp = [P, RB, K]
            NE = RB * K

            def wt(tag):
                return wpool.tile(shp, F32, tag=tag, name=tag)

            def floor_chain(dst, src_ap):
                nc.vector.tensor_scalar(
                    out=dst[:], in0=src_ap, scalar1=-2.0, scalar2=-3.0,
                    op0=A_OP.is_ge, op1=A_OP.add)
                for t in (-1.0, 0.0, 1.0, 2.0):
                    cmp_t = wt("cmp")
                    nc.vector.tensor_scalar(
                        out=cmp_t[:], in0=src_ap, scalar1=t, scalar2=None,
                        op0=A_OP.is_ge)
                    nc.vector.tensor_tensor(
                        out=dst[:], in0=dst[:], in1=cmp_t[:], op=A_OP.add)

            def bound_mask(dst, src, lo, hi):
                m2 = wt("mtmp")
                nc.vector.tensor_scalar(out=dst[:], in0=src[:], scalar1=lo,
                                        scalar2=None, op0=A_OP.is_ge)
                nc.vector.tensor_scalar(out=m2[:], in0=src[:], scalar1=hi,
                                        scalar2=None, op0=A_OP.is_le)
                nc.vector.tensor_tensor(out=dst[:], in0=dst[:], in1=m2[:],
                                        op=A_OP.mult)

            part_dim = offT[:].ap[0]
            base_off = offT[:].offset
            for br in range(NBR):
                # strided views of offT: channel (br*18 + 2k) (+1 for x-offset)
                offy_v = AP(offT.tensor, base_off + br * 32,
                            [part_dim, [96, RB], [2, K]])
                offx_v = AP(offT.tensor, base_off + br * 32 + 1,
                            [part_dim, [96, RB], [2, K]])
                cst = lambda t: t[:, br * NE:(br + 1) * NE].rearrange(
                    "p (r k) -> p r k", k=K)
                ybrel_v = cst(ybrel)
                ybabs_v = cst(ybabs)
                xvb_v = cst(xvb)

                fy = wt("fy")
                floor_chain(fy, offy_v)
                fx = wt("fx")
                floor_chain(fx, offx_v)

                ay = wt("ay")
                nc.vector.tensor_tensor(out=ay[:], in0=offy_v, in1=fy[:],
                                        op=A_OP.subtract)
                ax = wt("ax")
                nc.vector.tensor_tensor(out=ax[:], in0=offx_v, in1=fx[:],
                                        op=A_OP.subtract)
                y0a = wt("y0a")
                nc.vector.tensor_tensor(out=y0a[:], in0=ybabs_v, in1=fy[:],
                                        op=A_OP.add)
                x0a = wt("x0a")
                nc.vector.tensor_tensor(out=x0a[:], in0=xvb_v, in1=fx[:],
                                        op=A_OP.add)

                msk = wt("msk")
                wy0 = wt("wy0")
                bound_mask(msk, y0a, 0.0, float(H - 1))
                nc.vector.tensor_scalar(out=wy0[:], in0=ay[:], scalar1=-1.0,
                                        scalar2=1.0, op0=A_OP.mult, op1=A_OP.add)
                nc.vector.tensor_tensor(out=wy0[:], in0=wy0[:], in1=msk[:],
                                        op=A_OP.mult)
                msk2 = wt("msk2")
                wy1 = wt("wy1")
                bound_mask(msk2, y0a, -1.0, float(H - 2))
                nc.vector.tensor_tensor(out=wy1[:], in0=ay[:], in1=msk2[:],
                                        op=A_OP.mult)
                msk3 = wt("msk3")
                wx0 = wt("wx0")
                bound_mask(msk3, x0a, 0.0, float(W - 1))
                nc.vector.tensor_scalar(out=wx0[:], in0=ax[:], scalar1=-1.0,
                                        scalar2=1.0, op0=A_OP.mult, op1=A_OP.add)
                nc.vector.tensor_tensor(out=wx0[:], in0=wx0[:], in1=msk3[:],
                                        op=A_OP.mult)
                msk4 = wt("msk4")
                wx1 = wt("wx1")
                bound_mask(msk4, x0a, -1.0, float(W - 2))
                nc.vector.tensor_tensor(out=wx1[:], in0=ax[:], in1=msk4[:],
                                        op=A_OP.mult)

                nc.vector.tensor_tensor(out=s00[:, br], in0=wy0[:], in1=wx0[:],
                                        op=A_OP.mult)
                nc.vector.tensor_tensor(out=s01[:, br], in0=wy0[:], in1=wx1[:],
                                        op=A_OP.mult)
                nc.vector.tensor_tensor(out=s10[:, br], in0=wy1[:], in1=wx0[:],
                                        op=A_OP.mult)
                nc.vector.tensor_tensor(out=s11[:, br], in0=wy1[:], in1=wx1[:],
                                        op=A_OP.mult)

                idxf = wt("idxf")
                nc.vector.tensor_tensor(out=idxf[:], in0=ybrel_v, in1=fy[:],
                                        op=A_OP.add)
                nc.vector.tensor_scalar(out=idxf[:], in0=idxf[:], scalar1=float(W),
                                        scalar2=None, op0=A_OP.mult)
                nc.vector.tensor_tensor(out=idxf[:], in0=idxf[:], in1=x0a[:],
                                        op=A_OP.add)
                nc.vector.tensor_copy(idx0[:, br], idxf[:])

            if DEBUG_TAPS:
                nc.sync.dma_start(d_dbg_offT[:],
                                  offT[:].rearrange("p a b -> p (a b)"))
                for i, st in enumerate((s00, s01, s10, s11)):
                    nc.sync.dma_start(
                        d_dbg_s[:, i * NBR * RB * K:(i + 1) * NBR * RB * K],
                        st[:].rearrange("p a b c -> p (a b c)"))
                nc.sync.dma_start(
                    d_dbg_idx[:, 0:NBR * RB * K],
                    idx0[:].rearrange("p a b c -> p (a b c)"))

            xT_view = d_xT[:]
            _qctr = [0]

            def gelu_chain(y_ap, n, tag):
                u = mpool.tile([P, n], BF16, tag=tag + "u")
                nc.vector.tensor_tensor(out=u[:], in0=y_ap, in1=y_ap, op=A_OP.mult)
                nc.vector.tensor_scalar(out=u[:], in0=u[:], scalar1=G_B,
                                        scalar2=G_A, op0=A_OP.mult, op1=A_OP.add)
                nc.vector.tensor_tensor(out=u[:], in0=u[:], in1=y_ap, op=A_OP.mult)
                s = mpool.tile([P, n], BF16, tag=tag + "s")
                nc.scalar.activation(out=s[:], in_=u[:], func=AF.Sigmoid)
                nc.any.tensor_tensor(out=y_ap, in0=y_ap, in1=s[:], op=A_OP.mult)

            def rsqrt_dve(var_t, n, tag):
                # Newton rsqrt (no ACT table): y0 via bit trick, 3 iterations
                vi = mpool.tile([P, n], I32, tag="rsvi", name="rsvi", padded_shape=[P, 3])
                nc.vector.tensor_scalar(
                    out=vi[:], in0=var_t[:].bitcast(I32), scalar1=1,
                    scalar2=None, op0=A_OP.arith_shift_right)
                nc.vector.tensor_scalar(
                    out=vi[:], in0=vi[:], scalar1=-1, scalar2=0x5f3759df,
                    op0=A_OP.mult, op1=A_OP.add)
                y = mpool.tile([P, n], F32, tag="rsy", name="rsy", padded_shape=[P, 3])
                nc.vector.tensor_copy(y[:], vi[:].bitcast(F32))
                t2 = mpool.tile([P, n], F32, tag="rst2", name="rst2", padded_shape=[P, 3])
                for _ in range(3):
                    nc.vector.tensor_tensor(out=t2[:], in0=y[:], in1=y[:],
                                            op=A_OP.mult)
                    nc.vector.tensor_tensor(out=t2[:], in0=t2[:], in1=var_t[:],
                                            op=A_OP.mult)
                    nc.vector.tensor_scalar(out=t2[:], in0=t2[:], scalar1=-0.5,
                                            scalar2=1.5, op0=A_OP.mult,
                                            op1=A_OP.add)
                    nc.vector.tensor_tensor(out=y[:], in0=y[:], in1=t2[:],
                                            op=A_OP.mult)
                return y

            def ln_stats(src_ap, n, ngrp, tag):
                # src [P, ngrp, n] f32 view -> (mu [P, ngrp], rstd [P, ngrp])
                mu = mpool.tile([P, ngrp], F32, tag="lnmu", name="lnmu", padded_shape=[P, 3])
                nc.vector.tensor_reduce(
                    out=mu[:], in_=src_ap, op=A_OP.add, axis=mybir.AxisListType.X)
                nc.vector.tensor_scalar(out=mu[:], in0=mu[:], scalar1=1.0 / n,
                                        scalar2=None, op0=A_OP.mult)
                sq = mpool.tile([P, ngrp * n], F32, tag="lnsq", name="lnsq", padded_shape=[P, NBR * C])
                nc.vector.tensor_tensor(out=sq[:], in0=src_ap, in1=src_ap,
                                        op=A_OP.mult)
                ssq = mpool.tile([P, ngrp], F32, tag="lnssq", name="lnssq", padded_shape=[P, 3])
                nc.vector.tensor_reduce(
                    out=ssq[:], in_=sq[:].rearrange("p (g n) -> p g n", n=n),
                    op=A_OP.add, axis=mybir.AxisListType.X)
                var = mpool.tile([P, ngrp], F32, tag="lnvar", name="lnvar", padded_shape=[P, 3])
                nc.vector.tensor_tensor(out=var[:], in0=mu[:], in1=mu[:],
                                        op=A_OP.mult)
                nc.vector.scalar_tensor_tensor(
                    out=var[:], in0=ssq[:], scalar=1.0 / n, in1=var[:],
                    op0=A_OP.mult, op1=A_OP.subtract)
                nc.vector.tensor_scalar(out=var[:], in0=var[:], scalar1=EPS,
                                        scalar2=None, op0=A_OP.add)
                rstd = rsqrt_dve(var, ngrp, tag)
                return mu, rstd

            def ln_norm3(src_t, dst_t):
                # src [P, NBR, C] f32; dst [P, NBR*C] bf16
                mu, rstd = ln_stats(src_t[:], C, NBR, "ln3")
                for b3 in range(NBR):
                    nc.vector.tensor_scalar(
                        out=dst_t[:, b3 * C:(b3 + 1) * C], in0=src_t[:, b3, :],
                        scalar1=mu[:, b3:b3 + 1], scalar2=rstd[:, b3:b3 + 1],
                        op0=A_OP.subtract, op1=A_OP.mult)

            def ln_norm(src_ap, dst_ap, nfree, tag):
                mu, rstd = ln_stats(
                    src_ap.rearrange("p (g n) -> p g n", g=1), nfree, 1, tag)
                nc.vector.tensor_scalar(
                    out=dst_ap, in0=src_ap, scalar1=mu[:, 0:1], scalar2=rstd[:, 0:1],
                    op0=A_OP.subtract, op1=A_OP.mult)

            # ---- stage 4: per-row pipeline ----
            for r in range(RB):
                glu = apool.tile([P, NBR * C], BF16, tag="glu")
                for br in range(NBR):
                    g0 = gpool.tile([P, K, 4 * C], BF16, tag="g")
                    for k in range(K):
                        gi = nc.gpsimd.indirect_dma_start(
                            out=g0[:, k, :], out_offset=None, in_=xT_view,
                            in_offset=bass.IndirectOffsetOnAxis(
                                ap=idx0[:, br, r, k:k + 1], axis=0))
                        qn = _qctr[0] % 4
                        gi.queue = f"qPoolDynamic{qn if qn else ''}"
                        _qctr[0] += 1
                    if DEBUG_TAPS and r == 0 and br == 0:
                        nc.gpsimd.dma_start(d_dbg_g[:],
                                            g0[:].rearrange("p a b -> p (a b)"))
                    acc = apool.tile([P, K, C], BF16, tag="acc")
                    for k in range(K):
                        tp4k = apool.tile([P, 2, C], BF16, tag="tp4", bufs=6)
                        a_sl = acc[:, k, :]
                        # 4 tap products on DVE (single-src 4x mode)
                        nc.vector.tensor_scalar(
                            out=a_sl, in0=g0[:, k, 0:C],
                            scalar1=s00[:, br, r, k:k + 1], scalar2=None,
                            op0=A_OP.mult)
                        nc.vector.tensor_scalar(
                            out=tp4k[:, 0, :], in0=g0[:, k, C:2 * C],
                            scalar1=s10[:, br, r, k:k + 1], scalar2=None,
                            op0=A_OP.mult)
                        nc.vector.tensor_scalar(
                            out=tp4k[:, 1, :], in0=g0[:, k, 2 * C:3 * C],
                            scalar1=s01[:, br, r, k:k + 1], scalar2=None,
                            op0=A_OP.mult)
                        nc.any.tensor_tensor(out=a_sl, in0=a_sl,
                                             in1=tp4k[:, 0, :], op=A_OP.add)
                        nc.vector.tensor_scalar(
                            out=tp4k[:, 0, :], in0=g0[:, k, 3 * C:4 * C],
                            scalar1=s11[:, br, r, k:k + 1], scalar2=None,
                            op0=A_OP.mult)
                        nc.any.tensor_tensor(out=tp4k[:, 1, :],
                                             in1=tp4k[:, 0, :],
                                             in0=tp4k[:, 1, :], op=A_OP.add)
                        nc.any.tensor_tensor(out=a_sl, in0=a_sl,
                                             in1=tp4k[:, 1, :], op=A_OP.add)
                        nc.any.tensor_tensor(
                            out=a_sl, in0=a_sl,
                            in1=defw[:, (br * K + k) * C:(br * K + k + 1) * C],
                            op=A_OP.mult)
                    # k-sum on PE: 9 identity-matmuls accumulate into PSUM
                    ps_df = pspool.tile([P, C], F32, tag="psdf", bufs=2)
                    for k in range(K):
                        nc.tensor.matmul(ps_df[:], lhsT=identb[:],
                                         rhs=acc[:, k, :],
                                         start=(k == 0), stop=(k == K - 1))
                    ysum = apool.tile([P, C], F32, tag="ysum")
                    nc.any.tensor_copy(ysum[:], ps_df[:])
                    ln_norm(ysum[:], glu[:, br * C:(br + 1) * C], C, "lnb")
                    if DEBUG_TAPS and r == 0 and br == 0:
                        nc.sync.dma_start(d_dbg_acc[:], glu[:, 0:C])

                # batched gelu over all 3 branch outputs
                gelu_chain(glu[:], NBR * C, "glb")

                # 1x1 conv for this row
                c1 = mpool.tile([P, C], BF16, tag="c1")
                for ct in range(2):
                    ps_c = pspool.tile([P, P], F32, tag="ps_sm", bufs=1)
                    for kt in range(2):
                        nc.tensor.matmul(
                            ps_c[:],
                            lhsT=convw[:, (kt * 2 + ct) * P:(kt * 2 + ct + 1) * P],
                            rhs=xslab[kt][:, HALO + r, HALO:HALO + P],
                            start=(kt == 0), stop=(kt == 1))
                    cb = mpool.tile([P, P], BF16, tag="cb")
                    nc.vector.tensor_scalar(out=cb[:], in0=ps_c[:],
                                            scalar1=convb[:, ct:ct + 1],
                                            scalar2=None, op0=A_OP.add)
                    ps_ct = pspool.tile([P, P], BF16, tag="ps_smb")
                    nc.tensor.transpose(ps_ct[:], cb[:], identb[:])
                    nc.any.tensor_copy(c1[:, ct * P:(ct + 1) * P], ps_ct[:])

                if DEBUG_TAPS and r == 0:
                    nc.gpsimd.dma_start(d_dbg_glu[:], glu[:])
                tot = mpool.tile([P, C], F32, tag="tot")
                nc.any.tensor_tensor(out=tot[:], in0=glu[:, 0:C],
                                     in1=glu[:, C:2 * C], op=A_OP.add)
                nc.any.tensor_tensor(out=tot[:], in0=tot[:],
                                     in1=glu[:, 2 * C:3 * C], op=A_OP.add)
                nc.any.tensor_tensor(out=tot[:], in0=tot[:], in1=c1[:], op=A_OP.add)

                if DEBUG_TAPS and r == 0:
                    nc.sync.dma_start(d_dbg_tot[:], tot[:])
                outr = mpool.tile([P, C], BF16, tag="outr")
                ln_norm(tot[:], outr[:], C, "ln1")

                # MLP
                outT = mpool.tile([P, 2, P], F32, tag="outT")
                for ct in range(2):
                    ps_tr = pspool.tile([P, P], BF16, tag="ps_smb")
                    nc.tensor.transpose(ps_tr[:], outr[:, ct * P:(ct + 1) * P],
                                        identb[:])
                    nc.any.tensor_copy(outT[:, ct, :], ps_tr[:])

                ps_h = pspool.tile([P, 512], F32, tag="ps2k")
                for ct in range(2):
                    nc.tensor.matmul(ps_h[:], lhsT=outT[:, ct, :],
                                     rhs=w1T[:, ct * 512:(ct + 1) * 512],
                                     start=(ct == 0), stop=False)
                nc.tensor.matmul(ps_h[:], lhsT=ones1[:], rhs=b1row[:],
                                 start=False, stop=True)
                hx = mpool.tile([P, 512], BF16, tag="hx")
                nc.any.tensor_copy(hx[:], ps_h[:])
                hu = mpool.tile([P, 512], BF16, tag="hu")
                nc.vector.tensor_tensor(out=hu[:], in0=hx[:], in1=hx[:],
                                        op=A_OP.mult)
                nc.vector.tensor_scalar(out=hu[:], in0=hu[:], scalar1=G_B,
                                        scalar2=G_A, op0=A_OP.mult, op1=A_OP.add)
                nc.vector.tensor_tensor(out=hu[:], in0=hu[:], in1=hx[:],
                                        op=A_OP.mult)
                hs = mpool.tile([P, 512], BF16, tag="hs")
                nc.scalar.activation(out=hs[:], in_=hu[:], func=AF.Sigmoid)
                hg = mpool.tile([P, 512], BF16, tag="hg")
                nc.any.tensor_tensor(out=hg[:], in0=hx[:], in1=hs[:], op=A_OP.mult)

                if DEBUG_TAPS and r == 0:
                    nc.gpsimd.dma_start(d_dbg_h[:], hg[:])
                hT = mpool.tile([P, 4, P], F32, tag="hT")
                for jt in range(4):
                    ps_ht = pspool.tile([P, P], BF16, tag="ps_smb")
                    nc.tensor.transpose(ps_ht[:], hg[:, jt * P:(jt + 1) * P],
                                        identb[:])
                    nc.any.tensor_copy(hT[:, jt, :], ps_ht[:])

                ps_o = pspool.tile([P, C], F32, tag="pso", bufs=1)
                for jt in range(4):
                    nc.tensor.matmul(ps_o[:], lhsT=hT[:, jt, :],
                                     rhs=w2T[:, jt * C:(jt + 1) * C],
                                     start=(jt == 0), stop=False)
                nc.tensor.matmul(ps_o[:], lhsT=ones1[:], rhs=b2row[:],
                                 start=False, stop=True)

                res = mpool.tile([P, C], F32, tag="tot", name="res")
                nc.any.tensor_tensor(out=res[:], in0=ps_o[:], in1=outr[:],
                                     op=A_OP.add)
                orow = mpool.tile([P, C], F32, tag="orow")
                ln_norm(res[:], orow[:], C, "ln2")
                nc.sync.dma_start(d_out[r * P:(r + 1) * P, :], orow[:])

    nc.compile()
    return nc


def stage_inputs(inputs):
    """Build per-core in_maps from full inputs (layout/dtype staging only)."""
    x = np.asarray(inputs["x"], np.float32)
    off_w = [np.asarray(inputs[f"off_w{i}"], np.float32) for i in (1, 2, 3)]
    def_w = [np.asarray(inputs[f"def_w{i}"], np.float32) for i in (1, 2, 3)]
    conv_w = np.asarray(inputs["conv_w"], np.float32)[:, :, 0, 0]
    conv_b = np.asarray(inputs["conv_b"], np.float32)
    w1 = np.asarray(inputs["mlp_w1"], np.float32)
    b1 = np.asarray(inputs["mlp_b1"], np.float32)
    w2 = np.asarray(inputs["mlp_w2"], np.float32)
    b2 = np.asarray(inputs["mlp_b2"], np.float32)

    bf = lambda a: np.ascontiguousarray(a, np.float32).astype(mybir.dt.np(BF16))

    offw = np.zeros((NBR, K, 2, P, 18), np.float32)
    for br in range(NBR):
        for tap in range(K):
            ky, kx = tap // 3, tap % 3
            for kt in range(2):
                offw[br, tap, kt] = off_w[br][:, kt * P:(kt + 1) * P, ky, kx].T
    offw_f = offw.transpose(3, 0, 1, 2, 4).reshape(P, NBR * K * 2 * 18)

    convw = np.zeros((2, 2, P, P), np.float32)
    for kt in range(2):
        for ct in range(2):
            convw[kt, ct] = conv_w[ct * P:(ct + 1) * P, kt * P:(kt + 1) * P].T
    convw_f = convw.transpose(2, 0, 1, 3).reshape(P, 4 * P)

    convb_f = np.stack([conv_b[:P], conv_b[P:]], axis=1)

    defw = np.zeros((NBR, K, P, C), np.float32)
    for br in range(NBR):
        dwk = def_w[br][:, 0].reshape(C, K)
        for k in range(K):
            defw[br, k] = np.broadcast_to(dwk[:, k][None, :], (P, C))
    defw_f = defw.transpose(2, 0, 1, 3).reshape(P, NBR * K * C)

    w1T_f = np.concatenate([w1.T[:P], w1.T[P:]], axis=1)        # [128, 2*512]
    w2T_f = np.concatenate([w2.T[jt * P:(jt + 1) * P] for jt in range(4)], axis=1)

    identf = np.eye(P, dtype=np.float32)
    ones1 = np.ones((1, P), np.float32)

    shared = dict(
        offw=bf(offw_f), convw=bf(convw_f), convb=convb_f, defw=bf(defw_f),
        w1T=w1T_f, b1row=b1[None, :].copy(), w2T=w2T_f, b2row=b2[None, :].copy(),
        identf=identf, identb=bf(identf), ones1=ones1,
    )

    in_maps = []
    xr = x.reshape(B, 2, P, H, W)
    for core in range(NCORES):
        b = core // 4
        y0 = (core % 4) * RB
        slab = np.zeros((2, P, SROWS, WP), np.float32)
        rlo, rhi = y0 - HALO, y0 + RB + HALO
        srlo, srhi = max(rlo, 0), min(rhi, H)
        slab[:, :, srlo - rlo:srhi - rlo, HALO:HALO + W] = xr[b][:, :, srlo:srhi]
        xT1 = np.zeros((TR_ALLOC + W, C), np.float32)
        tlo = y0 - TPAD
        alo, ahi = max(tlo, 0), min(y0 + RB + TPAD + 1, H)
        xT1[(alo - tlo) * W:(ahi - tlo) * W] = \
            x[b, :, alo:ahi, :].reshape(C, -1).T
        xT = np.concatenate([xT1[:TR_ALLOC], xT1[W:TR_ALLOC + W]], axis=1)
        ybrel = np.zeros((P, NBR, RB, K), np.float32)
        ybabs = np.zeros((P, NBR, RB, K), np.float32)
        xvb = np.zeros((P, NBR, RB, K), np.float32)
        for br in range(NBR):
            dil = DILS[br]
            for k in range(K):
                ky, kx = k // 3, k % 3
                rows = y0 + np.arange(RB) + (ky - 1) * dil
                ybabs[:, br, :, k] = rows[None, :]
                ybrel[:, br, :, k] = rows[None, :] - y0 + TPAD
                xvb[:, br, :, k] = (np.arange(P) + (kx - 1) * dil)[:, None]
        m = dict(shared)
        m.update(xslab=bf(slab.reshape(2, P, SROWS * WP)), xT=bf(xT),
                 ybrel=bf(ybrel.reshape(P, -1)), ybabs=bf(ybabs.reshape(P, -1)),
                 xvb=bf(xvb.reshape(P, -1)))
        in_maps.append(m)
    return in_maps


def assemble_output(results):
    out = np.zeros((B, C, H, W), np.float32)
    for core in range(NCORES):
        b = core // 4
        y0 = (core % 4) * RB
        o = np.asarray(results[core]["out"], np.float32)
        out[b, :, y0:y0 + RB, :] = o.reshape(RB, W, C).transpose(2, 0, 1)
    return out


def kernel(**inputs):
    global _COMPILED
    from concourse.bass_utils import run_bass_kernel_spmd
    if _COMPILED is None:
        _COMPILED = build_program()
    nc = _COMPILED
    in_maps = stage_inputs(inputs)
    res = run_bass_kernel_spmd(nc, in_maps, core_ids=list(range(NCORES)))
    return assemble_output(res.results)



# revision 6
# speedup vs baseline: 1.1048x; 1.1048x over previous
"""Trainium2 Bass kernel for nn_DetailCapture (deformable-conv detail-capture block).

Sharding: 8 cores = batch (2) x row-blocks (4 x 32 rows). Each core computes its
[b, :, y0:y0+32, :] output slice from host-staged per-core input slabs (halos are
baked into the slabs, so no collectives are needed).

Per-core device pipeline (per 32-row block):
  1. Offset convs for all 3 branches via PE matmuls (9 taps x 2 k-tiles, PSUM acc)
  2. PE-transpose offsets to [pixel, 54] layout
  3. Floor/frac/bilinear-weight/index tables on DVE (is_ge floor chains)
  4. Per row x branch: ONE batched indirect-DMA gather of all 9 taps (4 bilinear
     values per index: 2 contiguous x-taps per xT row, y-taps via idx and idx+W)
  5. Bilinear apply: ACT first-tap product + DVE scalar_tensor_tensor accumulate
     chain; depthwise weights via one big tensor_tensor; k-sum on PE (identity
     matmuls into PSUM); LayerNorm2d stats via ACT Square/accum_out; exact Gelu
     on ACT fused with the LN normalize
  6. 1x1 conv (PE, bias preloaded in PSUM), branch sum, LN, per-pixel MLP
     (bf16 PE matmuls, ACT Gelu), residual, LN
"""
import sys
import numpy as np

sys.path.insert(0, "/opt/trn_rl_repo")

import concourse.bass as bass
import concourse.bacc as bacc
import concourse.mybir as mybir
import concourse.tile as tile
from concourse.bass import AP

P = 128
B, C, H, W = 2, 256, 128, 128
RB = 32                # rows per core block
NCORES = 8
DILS = (1, 9, 12)
NBR = 3
K = 9
HALO = 12              # conv halo (max dil)
SROWS = RB + 2 * HALO  # 56 rows in conv slab
WP = W + 2 * HALO      # 152 padded width
TPAD = 16              # xT slab vertical pad rows (each side)
TROWS = (RB + 2 * TPAD) * W
TR_ALLOC = TROWS + 2 * W
A_OP = mybir.AluOpType
F32 = mybir.dt.float32
BF16 = mybir.dt.bfloat16
I32 = mybir.dt.int32
AF = mybir.ActivationFunctionType

EPS = 1e-6

# engine-balance knobs
GPS_K = ()            # set of (br, k) chains routed to gpsimd
GPS_DEFW = 0          # how many of the 3 per-row defw multiplies go to gpsimd

_COMPILED = None


def build_program():
    nc = bacc.Bacc(None, target_bir_lowering=False, debug=False,
                   num_swdge_queues=4)

    # ---------------- DRAM I/O (host-staged layouts) ----------------
    d_xslab = nc.dram_tensor("xslab", [2, P, SROWS * WP], BF16, kind="ExternalInput")
    d_xT = nc.dram_tensor("xT", [TR_ALLOC, 2 * C], BF16, kind="ExternalInput")
    d_offw = nc.dram_tensor("offw", [P, NBR * K * 2 * 18], BF16, kind="ExternalInput")
    d_convw = nc.dram_tensor("convw", [P, 4 * P], BF16, kind="ExternalInput")
    d_convb = nc.dram_tensor("convb", [1, 2 * P], BF16, kind="ExternalInput")
    d_defw = nc.dram_tensor("defw", [P, NBR * K * C], BF16, kind="ExternalInput")
    d_w1T = nc.dram_tensor("w1T", [P, 2 * 512], BF16, kind="ExternalInput")
    d_b1row = nc.dram_tensor("b1row", [1, 512], BF16, kind="ExternalInput")
    d_w2T = nc.dram_tensor("w2T", [P, 4 * C], BF16, kind="ExternalInput")
    d_b2row = nc.dram_tensor("b2row", [1, C], BF16, kind="ExternalInput")
    d_ybrel = nc.dram_tensor("ybrel", [P, NBR * RB * K], BF16, kind="ExternalInput")
    d_ybabs = nc.dram_tensor("ybabs", [P, NBR * RB * K], BF16, kind="ExternalInput")
    d_xvb = nc.dram_tensor("xvb", [P, NBR * RB * K], BF16, kind="ExternalInput")
    d_ident = nc.dram_tensor("identf", [P, P], F32, kind="ExternalInput")
    d_identb = nc.dram_tensor("identb", [P, P], BF16, kind="ExternalInput")
    d_ones1 = nc.dram_tensor("ones1", [1, P], BF16, kind="ExternalInput")
    d_out = nc.dram_tensor("out", [RB * P, C], F32, kind="ExternalOutput")

    with tile.TileContext(nc) as tc:
        import contextlib
        ctx = contextlib.ExitStack()
        with ctx:
            cpool = ctx.enter_context(tc.tile_pool(name="const", bufs=1))
            spool = ctx.enter_context(tc.tile_pool(name="slab", bufs=1))
            tpool = ctx.enter_context(tc.tile_pool(name="tables", bufs=1))
            wpool = ctx.enter_context(tc.tile_pool(name="work", bufs=1))
            gpool = ctx.enter_context(tc.tile_pool(name="gath", bufs=2))
            apool = ctx.enter_context(tc.tile_pool(name="apply", bufs=2))
            mpool = ctx.enter_context(tc.tile_pool(name="mlp", bufs=2))
            pspool = ctx.enter_context(tc.tile_pool(name="ps", bufs=2, space="PSUM"))

            # ---- constants ----
            def load_const(name, dram, shape, dtype):
                t = cpool.tile(shape, dtype, tag=name, name=name)
                nc.sync.dma_start(t[:], dram[:])
                return t

            ident = load_const("ident", d_ident, [P, P], F32)
            identb = load_const("identb", d_identb, [P, P], BF16)
            ones1 = load_const("ones1", d_ones1, [1, P], BF16)
            convb = load_const("convb", d_convb, [1, 2 * P], BF16)
            offw = load_const("offw", d_offw, [P, NBR * K * 2 * 18], BF16)
            convw = load_const("convw", d_convw, [P, 4 * P], BF16)
            defw = load_const("defw", d_defw, [P, NBR * K * C], BF16)
            w1T = load_const("w1T", d_w1T, [P, 2 * 512], BF16)
            w2T = load_const("w2T", d_w2T, [P, 4 * C], BF16)
            b1row = load_const("b1row", d_b1row, [1, 512], BF16)
            b2row = load_const("b2row", d_b2row, [1, C], BF16)
            ybrel = load_const("ybrel", d_ybrel, [P, NBR * RB * K], BF16)
            ybabs = load_const("ybabs", d_ybabs, [P, NBR * RB * K], BF16)
            xvb = load_const("xvb", d_xvb, [P, NBR * RB * K], BF16)

            xslab = [spool.tile([P, SROWS, WP], BF16, tag=f"xs{kt}", name=f"xs{kt}") for kt in range(2)]
            for kt in range(2):
                nc.sync.dma_start(
                    xslab[kt][:], d_xslab[kt].rearrange("p (r w) -> p r w", w=WP))

            def offw_ap(br, tap, kt):
                base = ((br * K + tap) * 2 + kt) * 18
                return offw[:, base:base + 18]

            # ---- stage 1+2: offset convs -> transpose -> offT [128, RB, 96] ----
            # (each branch's 18 offset channels at partition base 32*br: PE
            #  output base partitions must be 32-aligned)
            offT = tpool.tile([P, RB, 96], F32)
            nc.vector.memset(offT[:], 0.0)
            for chunk in range(RB // 4):          # 4 rows = 512 px per chunk
                ps_off = pspool.tile([96, 512], F32, tag="ps2k", bufs=2)
                for br in range(NBR):
                    dil = DILS[br]
                    for tap in range(K):
                        dy, dx = tap // 3, tap % 3
                        srow = 4 * chunk + HALO + (dy - 1) * dil
                        scol = HALO + (dx - 1) * dil
                        for kt in range(2):
                            nc.tensor.matmul(
                                ps_off[br * 32:br * 32 + 18, :],
                                lhsT=offw_ap(br, tap, kt),
                                rhs=xslab[kt][:, srow:srow + 4, scol:scol + P],
                                start=(tap == 0 and kt == 0),
                                stop=(tap == K - 1 and kt == 1),
                            )
                osb = wpool.tile([96, 512], F32, tag="osb")
                for br in range(NBR):
                    nc.any.tensor_copy(osb[br * 32:br * 32 + 18, :],
                                       ps_off[br * 32:br * 32 + 18, :])
                for sub in range(4):
                    r = chunk * 4 + sub
                    for br in range(NBR):
                        ps_t = pspool.tile([P, 18], F32, tag="ps_sm", bufs=1)
                        nc.tensor.transpose(
                            ps_t[:], osb[br * 32:br * 32 + 18, sub * P:(sub + 1) * P],
                            ident[br * 32:br * 32 + 18, br * 32:br * 32 + 18])
                        nc.any.tensor_copy(offT[:, r, br * 32:br * 32 + 18], ps_t[:])

            # ---- stage 3: per-branch tables ----
            s00 = tpool.tile([P, NBR, RB, K], F32)
            s01 = tpool.tile([P, NBR, RB, K], F32)
            s10 = tpool.tile([P, NBR, RB, K], F32)
            s11 = tpool.tile([P, NBR, RB, K], F32)
            idx0 = tpool.tile([P, NBR, RB, K], I32)

            shp = [P, RB, K]
            NE = RB * K

            def wt(tag):
                return wpool.tile(shp, F32, tag=tag, name=tag)

            def floor_chain(dst, src_ap):
                nc.vector.tensor_scalar(
                    out=dst[:], in0=src_ap, scalar1=-2.0, scalar2=-3.0,
                    op0=A_OP.is_ge, op1=A_OP.add)
                for t in (-1.0, 0.0, 1.0, 2.0):
                    cmp_t = wt("cmp")
                    nc.vector.tensor_scalar(
                        out=cmp_t[:], in0=src_ap, scalar1=t, scalar2=None,
                        op0=A_OP.is_ge)
                    nc.vector.tensor_tensor(
                        out=dst[:], in0=dst[:], in1=cmp_t[:], op=A_OP.add)

            def bound_mask(dst, src, lo, hi):
                m2 = wt("mtmp")
                nc.vector.tensor_scalar(out=dst[:], in0=src[:], scalar1=lo,
                                        scalar2=None, op0=A_OP.is_ge)
                nc.vector.tensor_scalar(out=m2[:], in0=src[:], scalar1=hi,
                                        scalar2=None, op0=A_OP.is_le)
                nc.vector.tensor_tensor(out=dst[:], in0=dst[:], in1=m2[:],
                                        op=A_OP.mult)

            part_dim = offT[:].ap[0]
            base_off = offT[:].offset
            for br in range(NBR):
                # strided views of offT: channel (br*18 + 2k) (+1 for x-offset)
                offy_v = AP(offT.tensor, base_off + br * 32,
                            [part_dim, [96, RB], [2, K]])
                offx_v = AP(offT.tensor, base_off + br * 32 + 1,
                            [part_dim, [96, RB], [2, K]])
                cst = lambda t: t[:, br * NE:(br + 1) * NE].rearrange(
                    "p (r k) -> p r k", k=K)
                ybrel_v = cst(ybrel)
                ybabs_v = cst(ybabs)
                xvb_v = cst(xvb)

                fy = wt("fy")
                floor_chain(fy, offy_v)
                fx = wt("fx")
                floor_chain(fx, offx_v)

                ay = wt("ay")
                nc.vector.tensor_tensor(out=ay[:], in0=offy_v, in1=fy[:],
                                        op=A_OP.subtract)
                ax = wt("ax")
                nc.vector.tensor_tensor(out=ax[:], in0=offx_v, in1=fx[:],
                                        op=A_OP.subtract)
                y0a = wt("y0a")
                nc.vector.tensor_tensor(out=y0a[:], in0=ybabs_v, in1=fy[:],
                                        op=A_OP.add)
                x0a = wt("x0a")
                nc.vector.tensor_tensor(out=x0a[:], in0=xvb_v, in1=fx[:],
                                        op=A_OP.add)

                msk = wt("msk")
                wy0 = wt("wy0")
                bound_mask(msk, y0a, 0.0, float(H - 1))
                nc.vector.tensor_scalar(out=wy0[:], in0=ay[:], scalar1=-1.0,
                                        scalar2=1.0, op0=A_OP.mult, op1=A_OP.add)
                nc.vector.tensor_tensor(out=wy0[:], in0=wy0[:], in1=msk[:],
                                        op=A_OP.mult)
                msk2 = wt("msk2")
                wy1 = wt("wy1")
                bound_mask(msk2, y0a, -1.0, float(H - 2))
                nc.vector.tensor_tensor(out=wy1[:], in0=ay[:], in1=msk2[:],
                                        op=A_OP.mult)
                msk3 = wt("msk3")
                wx0 = wt("wx0")
                bound_mask(msk3, x0a, 0.0, float(W - 1))
                nc.vector.tensor_scalar(out=wx0[:], in0=ax[:], scalar1=-1.0,
                                        scalar2=1.0, op0=A_OP.mult, op1=A_OP.add)
                nc.vector.tensor_tensor(out=wx0[:], in0=wx0[:], in1=msk3[:],
                                        op=A_OP.mult)
                msk4 = wt("msk4")
                wx1 = wt("wx1")
                bound_mask(msk4, x0a, -1.0, float(W - 2))
                nc.vector.tensor_tensor(out=wx1[:], in0=ax[:], in1=msk4[:],
                                        op=A_OP.mult)

                nc.vector.tensor_tensor(out=s00[:, br], in0=wy0[:], in1=wx0[:],
                                        op=A_OP.mult)
                nc.vector.tensor_tensor(out=s01[:, br], in0=wy0[:], in1=wx1[:],
                                        op=A_OP.mult)
                nc.vector.tensor_tensor(out=s10[:, br], in0=wy1[:], in1=wx0[:],
                                        op=A_OP.mult)
                nc.vector.tensor_tensor(out=s11[:, br], in0=wy1[:], in1=wx1[:],
                                        op=A_OP.mult)

                idxf = wt("idxf")
                nc.vector.tensor_tensor(out=idxf[:], in0=ybrel_v, in1=fy[:],
                                        op=A_OP.add)
                nc.vector.tensor_scalar(out=idxf[:], in0=idxf[:], scalar1=float(W),
                                        scalar2=None, op0=A_OP.mult)
                nc.vector.tensor_tensor(out=idxf[:], in0=idxf[:], in1=x0a[:],
                                        op=A_OP.add)
                nc.vector.tensor_copy(idx0[:, br], idxf[:])

            xT_view = d_xT[:]
            _qctr = [0]

            # ---- per-LN helper: mu/rstd from (sumx, ssq) [P, n] ----
            lnp = wpool

            def ln_murstd(sumx, ssq, n, ngrp, tag):
                mu = lnp.tile([P, ngrp], F32, tag=tag + "mu", name=tag + "mu",
                              padded_shape=[P, 4])
                nc.vector.tensor_scalar(out=mu[:], in0=sumx[:], scalar1=1.0 / n,
                                        scalar2=None, op0=A_OP.mult)
                mu2 = lnp.tile([P, ngrp], F32, tag=tag + "mu2", name=tag + "mu2",
                               padded_shape=[P, 4])
                nc.vector.tensor_tensor(out=mu2[:], in0=mu[:], in1=mu[:],
                                        op=A_OP.mult)
                var = lnp.tile([P, ngrp], F32, tag=tag + "var", name=tag + "var",
                               padded_shape=[P, 4])
                nc.vector.scalar_tensor_tensor(
                    out=var[:], in0=ssq[:], scalar=1.0 / n, in1=mu2[:],
                    op0=A_OP.mult, op1=A_OP.subtract)
                nc.vector.tensor_scalar(out=var[:], in0=var[:], scalar1=EPS,
                                        scalar2=None, op0=A_OP.add)
                sd = lnp.tile([P, ngrp], F32, tag=tag + "sd", name=tag + "sd",
                              padded_shape=[P, 4])
                nc.scalar.activation(out=sd[:], in_=var[:], func=AF.Sqrt)
                rstd = lnp.tile([P, ngrp], F32, tag=tag + "rs", name=tag + "rs",
                                padded_shape=[P, 4])
                nc.vector.reciprocal(out=rstd[:], in_=sd[:])
                return mu, rstd

            # ---- stage 4: per-row pipeline ----
            for r in range(RB):
                glu = apool.tile([P, NBR * C], BF16, tag="glu")
                sumx3 = wpool.tile([P, NBR], F32, tag="sumx3", name="sumx3",
                                   padded_shape=[P, 4])
                ssq3 = wpool.tile([P, NBR], F32, tag="ssq3", name="ssq3",
                                  padded_shape=[P, 4])
                ysums = []
                for br in range(NBR):
                    g0 = gpool.tile([P, K, 4 * C], BF16, tag="g", bufs=2)
                    for k in range(K):
                        gi = nc.gpsimd.indirect_dma_start(
                            out=g0[:, k, :], out_offset=None, in_=xT_view,
                            in_offset=bass.IndirectOffsetOnAxis(
                                ap=idx0[:, br, r, k:k + 1], axis=0))
                        qn = _qctr[0] % 4
                        gi.queue = f"qPoolDynamic{qn if qn else ''}"
                        _qctr[0] += 1

                    acc = apool.tile([P, K, C], BF16, tag="acc")
                    for k in range(K):
                        a_sl = acc[:, k, :]
                        eng = nc.gpsimd if (br, k) in GPS_K else None
                        if eng is None:
                            nc.scalar.activation(
                                out=a_sl, in_=g0[:, k, 0:C], func=AF.Copy,
                                scale=s00[:, br, r, k:k + 1])
                            eng = nc.vector
                        else:
                            eng.tensor_scalar(
                                out=a_sl, in0=g0[:, k, 0:C],
                                scalar1=s00[:, br, r, k:k + 1], scalar2=None,
                                op0=A_OP.mult)
                        eng.scalar_tensor_tensor(
                            out=a_sl, in0=g0[:, k, C:2 * C],
                            scalar=s10[:, br, r, k:k + 1], in1=a_sl,
                            op0=A_OP.mult, op1=A_OP.add)
                        eng.scalar_tensor_tensor(
                            out=a_sl, in0=g0[:, k, 2 * C:3 * C],
                            scalar=s01[:, br, r, k:k + 1], in1=a_sl,
                            op0=A_OP.mult, op1=A_OP.add)
                        eng.scalar_tensor_tensor(
                            out=a_sl, in0=g0[:, k, 3 * C:4 * C],
                            scalar=s11[:, br, r, k:k + 1], in1=a_sl,
                            op0=A_OP.mult, op1=A_OP.add)
                    # depthwise weights: one big multiply over all 9 taps
                    dw_eng = nc.gpsimd if br < GPS_DEFW else nc.vector
                    dw_eng.tensor_tensor(
                        out=acc[:].rearrange("p a b -> p (a b)"),
                        in0=acc[:].rearrange("p a b -> p (a b)"),
                        in1=defw[:, br * K * C:(br + 1) * K * C],
                        op=A_OP.mult)
                    # k-sum on PE: 9 identity-matmuls accumulate into PSUM
                    ps_df = pspool.tile([P, C], F32, tag="psdf", bufs=2)
                    for k in range(K):
                        nc.tensor.matmul(ps_df[:], lhsT=identb[:],
                                         rhs=acc[:, k, :],
                                         start=(k == 0), stop=(k == K - 1))
                    ysum = apool.tile([P, C], BF16, tag=f"ysum{br}",
                                      name=f"ysum{br}")
                    nc.scalar.activation(out=ysum[:], in_=ps_df[:], func=AF.Copy,
                                         accum_out=sumx3[:, br:br + 1])
                    sq = apool.tile([P, C], BF16, tag="sqscr", bufs=4)
                    nc.scalar.activation(out=sq[:], in_=ysum[:], func=AF.Square,
                                         accum_out=ssq3[:, br:br + 1])
                    ysums.append(ysum)

                mu3, rstd3 = ln_murstd(sumx3, ssq3, C, NBR, "l3")
                nmr3 = lnp.tile([P, NBR], F32, tag="nmr3", name="nmr3",
                                padded_shape=[P, 4])
                nc.vector.scalar_tensor_tensor(
                    out=nmr3[:], in0=mu3[:], scalar=-1.0, in1=rstd3[:],
                    op0=A_OP.mult, op1=A_OP.mult)
                for br in range(NBR):
                    nc.scalar.activation(
                        out=glu[:, br * C:(br + 1) * C], in_=ysums[br][:],
                        func=AF.Gelu, scale=rstd3[:, br:br + 1],
                        bias=nmr3[:, br:br + 1])

                # 1x1 conv for this row (bias preloaded into PSUM)
                c1 = mpool.tile([P, C], BF16, tag="c1")
                for ct in range(2):
                    ps_c = pspool.tile([P, P], F32, tag="ps_sm", bufs=1)
                    nc.tensor.matmul(
                        ps_c[:], lhsT=convb[:, ct * P:(ct + 1) * P],
                        rhs=ones1[:], start=True, stop=False)
                    for kt in range(2):
                        nc.tensor.matmul(
                            ps_c[:],
                            lhsT=convw[:, (kt * 2 + ct) * P:(kt * 2 + ct + 1) * P],
                            rhs=xslab[kt][:, HALO + r, HALO:HALO + P],
                            start=False, stop=(kt == 1))
                    cb = mpool.tile([P, P], BF16, tag="cb")
                    nc.scalar.activation(out=cb[:], in_=ps_c[:], func=AF.Copy)
                    ps_ct = pspool.tile([P, P], BF16, tag="ps_smb")
                    nc.tensor.transpose(ps_ct[:], cb[:], identb[:])
                    nc.scalar.activation(out=c1[:, ct * P:(ct + 1) * P],
                                         in_=ps_ct[:], func=AF.Copy)

                # branch sum + 1x1 conv + LN
                sum1 = wpool.tile([P, 1], F32, tag="sum1", name="sum1",
                                  padded_shape=[P, 4])
                ssq1 = wpool.tile([P, 1], F32, tag="ssq1", name="ssq1",
                                  padded_shape=[P, 4])
                tot = mpool.tile([P, C], BF16, tag="tot")
                nc.vector.tensor_tensor(out=tot[:], in0=glu[:, 0:C],
                                        in1=glu[:, C:2 * C], op=A_OP.add)
                nc.vector.tensor_tensor(out=tot[:], in0=tot[:],
                                        in1=glu[:, 2 * C:3 * C], op=A_OP.add)
                nc.vector.scalar_tensor_tensor(
                    out=tot[:], in0=c1[:], scalar=1.0, in1=tot[:],
                    op0=A_OP.mult, op1=A_OP.add, accum_out=sum1[:])
                sq1 = apool.tile([P, C], BF16, tag="sqscr", bufs=4)
                nc.scalar.activation(out=sq1[:], in_=tot[:], func=AF.Square,
                                     accum_out=ssq1[:])
                mu1, rstd1 = ln_murstd(sum1, ssq1, C, 1, "l1")
                outr = mpool.tile([P, C], BF16, tag="outr")
                nc.vector.tensor_scalar(
                    out=outr[:], in0=tot[:], scalar1=mu1[:, 0:1],
                    scalar2=rstd1[:, 0:1], op0=A_OP.subtract, op1=A_OP.mult)

                # MLP
                outT = mpool.tile([P, 2, P], BF16, tag="outT")
                for ct in range(2):
                    ps_tr = pspool.tile([P, P], BF16, tag="ps_smb")
                    nc.tensor.transpose(ps_tr[:], outr[:, ct * P:(ct + 1) * P],
                                        identb[:])
                    nc.scalar.activation(out=outT[:, ct, :], in_=ps_tr[:],
                                         func=AF.Copy)

                ps_h = pspool.tile([P, 512], F32, tag="ps2k")
                for ct in range(2):
                    nc.tensor.matmul(ps_h[:], lhsT=outT[:, ct, :],
                                     rhs=w1T[:, ct * 512:(ct + 1) * 512],
                                     start=(ct == 0), stop=False)
                nc.tensor.matmul(ps_h[:], lhsT=ones1[:], rhs=b1row[:],
                                 start=False, stop=True)
                hg = mpool.tile([P, 512], BF16, tag="hg")
                nc.scalar.activation(out=hg[:], in_=ps_h[:], func=AF.Gelu)

                hT = mpool.tile([P, 4, P], BF16, tag="hT")
                for jt in range(4):
                    ps_ht = pspool.tile([P, P], BF16, tag="ps_smb")
                    nc.tensor.transpose(ps_ht[:], hg[:, jt * P:(jt + 1) * P],
                                        identb[:])
                    nc.scalar.activation(out=hT[:, jt, :], in_=ps_ht[:],
                                         func=AF.Copy)

                ps_o = pspool.tile([P, C], F32, tag="pso", bufs=1)
                for jt in range(4):
                    nc.tensor.matmul(ps_o[:], lhsT=hT[:, jt, :],
                                     rhs=w2T[:, jt * C:(jt + 1) * C],
                                     start=(jt == 0), stop=False)
                nc.tensor.matmul(ps_o[:], lhsT=ones1[:], rhs=b2row[:],
                                 start=False, stop=True)

                # residual + LN
                sum2 = wpool.tile([P, 1], F32, tag="sum2", name="sum2",
                                  padded_shape=[P, 4])
                ssq2 = wpool.tile([P, 1], F32, tag="ssq2", name="ssq2",
                                  padded_shape=[P, 4])
                res = mpool.tile([P, C], F32, tag="res")
                nc.vector.scalar_tensor_tensor(
                    out=res[:], in0=ps_o[:], scalar=1.0, in1=outr[:],
                    op0=A_OP.mult, op1=A_OP.add, accum_out=sum2[:])
                sq2 = apool.tile([P, C], BF16, tag="sqscr", bufs=4)
                nc.scalar.activation(out=sq2[:], in_=res[:], func=AF.Square,
                                     accum_out=ssq2[:])
                mu2r, rstd2 = ln_murstd(sum2, ssq2, C, 1, "l2")
                orow = mpool.tile([P, C], F32, tag="orow")
                nc.vector.tensor_scalar(
                    out=orow[:], in0=res[:], scalar1=mu2r[:, 0:1],
                    scalar2=rstd2[:, 0:1], op0=A_OP.subtract, op1=A_OP.mult)
                nc.sync.dma_start(d_out[r * P:(r + 1) * P, :], orow[:])

    nc.compile()
    return nc


def stage_inputs(inputs):
    """Build per-core in_maps from full inputs (layout/dtype staging only)."""
    x = np.asarray(inputs["x"], np.float32)
    off_w = [np.asarray(inputs[f"off_w{i}"], np.float32) for i in (1, 2, 3)]
    def_w = [np.asarray(inputs[f"def_w{i}"], np.float32) for i in (1, 2, 3)]
    conv_w = np.asarray(inputs["conv_w"], np.float32)[:, :, 0, 0]
    conv_b = np.asarray(inputs["conv_b"], np.float32)
    w1 = np.asarray(inputs["mlp_w1"], np.float32)
    b1 = np.asarray(inputs["mlp_b1"], np.float32)
    w2 = np.asarray(inputs["mlp_w2"], np.float32)
    b2 = np.asarray(inputs["mlp_b2"], np.float32)

    bf = lambda a: np.ascontiguousarray(a, np.float32).astype(mybir.dt.np(BF16))

    offw = np.zeros((NBR, K, 2, P, 18), np.float32)
    for br in range(NBR):
        for tap in range(K):
            ky, kx = tap // 3, tap % 3
            for kt in range(2):
                offw[br, tap, kt] = off_w[br][:, kt * P:(kt + 1) * P, ky, kx].T
    offw_f = offw.transpose(3, 0, 1, 2, 4).reshape(P, NBR * K * 2 * 18)

    convw = np.zeros((2, 2, P, P), np.float32)
    for kt in range(2):
        for ct in range(2):
            convw[kt, ct] = conv_w[ct * P:(ct + 1) * P, kt * P:(kt + 1) * P].T
    convw_f = convw.transpose(2, 0, 1, 3).reshape(P, 4 * P)

    convb_f = conv_b[None, :].copy()     # [1, 256]

    defw = np.zeros((NBR, K, P, C), np.float32)
    for br in range(NBR):
        dwk = def_w[br][:, 0].reshape(C, K)
        for k in range(K):
            defw[br, k] = np.broadcast_to(dwk[:, k][None, :], (P, C))
    defw_f = defw.transpose(2, 0, 1, 3).reshape(P, NBR * K * C)

    w1T_f = np.concatenate([w1.T[:P], w1.T[P:]], axis=1)        # [128, 2*512]
    w2T_f = np.concatenate([w2.T[jt * P:(jt + 1) * P] for jt in range(4)], axis=1)

    identf = np.eye(P, dtype=np.float32)
    ones1 = np.ones((1, P), np.float32)

    shared = dict(
        offw=bf(offw_f), convw=bf(convw_f), convb=bf(convb_f), defw=bf(defw_f),
        w1T=bf(w1T_f), b1row=bf(b1[None, :]), w2T=bf(w2T_f), b2row=bf(b2[None, :]),
        identf=identf, identb=bf(identf), ones1=bf(ones1),
    )

    in_maps = []
    xr = x.reshape(B, 2, P, H, W)
    for core in range(NCORES):
        b = core // 4
        y0 = (core % 4) * RB
        slab = np.zeros((2, P, SROWS, WP), np.float32)
        rlo, rhi = y0 - HALO, y0 + RB + HALO
        srlo, srhi = max(rlo, 0), min(rhi, H)
        slab[:, :, srlo - rlo:srhi - rlo, HALO:HALO + W] = xr[b][:, :, srlo:srhi]
        xT1 = np.zeros((TR_ALLOC + W, C), np.float32)
        tlo = y0 - TPAD
        alo, ahi = max(tlo, 0), min(y0 + RB + TPAD + 1, H)
        xT1[(alo - tlo) * W:(ahi - tlo) * W] = \
            x[b, :, alo:ahi, :].reshape(C, -1).T
        xT = np.concatenate([xT1[:TR_ALLOC], xT1[W:TR_ALLOC + W]], axis=1)
        ybrel = np.zeros((P, NBR, RB, K), np.float32)
        ybabs = np.zeros((P, NBR, RB, K), np.float32)
        xvb = np.zeros((P, NBR, RB, K), np.float32)
        for br in range(NBR):
            dil = DILS[br]
            for k in range(K):
                ky, kx = k // 3, k % 3
                rows = y0 + np.arange(RB) + (ky - 1) * dil
                ybabs[:, br, :, k] = rows[None, :]
                ybrel[:, br, :, k] = rows[None, :] - y0 + TPAD
                xvb[:, br, :, k] = (np.arange(P) + (kx - 1) * dil)[:, None]
        m = dict(shared)
        m.update(xslab=bf(slab.reshape(2, P, SROWS * WP)), xT=bf(xT),
                 ybrel=bf(ybrel.reshape(P, -1)), ybabs=bf(ybabs.reshape(P, -1)),
                 xvb=bf(xvb.reshape(P, -1)))
        in_maps.append(m)
    return in_maps


def assemble_output(results):
    out = np.zeros((B, C, H, W), np.float32)
    for core in range(NCORES):
        b = core // 4
        y0 = (core % 4) * RB
        o = np.asarray(results[core]["out"], np.float32)
        out[b, :, y0:y0 + RB, :] = o.reshape(RB, W, C).transpose(2, 0, 1)
    return out


def kernel(**inputs):
    global _COMPILED
    from concourse.bass_utils import run_bass_kernel_spmd
    if _COMPILED is None:
        _COMPILED = build_program()
    nc = _COMPILED
    in_maps = stage_inputs(inputs)
    res = run_bass_kernel_spmd(nc, in_maps, core_ids=list(range(NCORES)))
    return assemble_output(res.results)


# revision 8
# speedup vs baseline: 1.3506x; 1.2224x over previous
"""Trainium2 Bass kernel for nn_DetailCapture (deformable-conv detail-capture block).

Sharding: 8 cores = batch (2) x row-blocks (4 x 32 rows). Each core computes its
[b, :, y0:y0+32, :] output slice from host-staged per-core input slabs (halos are
baked into the slabs, so no collectives are needed).

Per-core device pipeline (per 32-row block):
  1. Offset convs for all 3 branches via PE matmuls (9 taps x 2 k-tiles, PSUM acc)
  2. PE-transpose offsets to [pixel, 54] layout
  3. Floor/frac/bilinear-weight/index tables on DVE (is_ge floor chains)
  4. Per row x branch: ONE batched indirect-DMA gather of all 9 taps (4 bilinear
     values per index: 2 contiguous x-taps per xT row, y-taps via idx and idx+W)
  5. Bilinear apply: ACT first-tap product + DVE scalar_tensor_tensor accumulate
     chain; depthwise weights via one big tensor_tensor; k-sum on PE (identity
     matmuls into PSUM); LayerNorm2d stats via ACT Square/accum_out; exact Gelu
     on ACT fused with the LN normalize
  6. 1x1 conv (PE, bias preloaded in PSUM), branch sum, LN, per-pixel MLP
     (bf16 PE matmuls, ACT Gelu), residual, LN
"""
import sys
import numpy as np

sys.path.insert(0, "/opt/trn_rl_repo")

import concourse.bass as bass
import concourse.bacc as bacc
import concourse.mybir as mybir
import concourse.tile as tile
from concourse.bass import AP

P = 128
B, C, H, W = 2, 256, 128, 128
RB = 32                # rows per core block
NCORES = 8
DILS = (1, 9, 12)
NBR = 3
K = 9
HALO = 12              # conv halo (max dil)
SROWS = RB + 2 * HALO  # 56 rows in conv slab
WP = W + 2 * HALO      # 152 padded width
TPAD = 16              # xT slab vertical pad rows (each side)
TROWS = (RB + 2 * TPAD) * W
TR_ALLOC = TROWS + 2 * W
A_OP = mybir.AluOpType
F32 = mybir.dt.float32
BF16 = mybir.dt.bfloat16
I32 = mybir.dt.int32
AF = mybir.ActivationFunctionType

EPS = 1e-6

# engine-balance knobs
GPS_K = ()            # set of (br, k) chains routed to gpsimd
GPS_DEFW = 0          # how many of the 3 per-row defw multiplies go to gpsimd

_COMPILED = None


def build_program():
    nc = bacc.Bacc(None, target_bir_lowering=False, debug=False,
                   num_swdge_queues=4)

    # ---------------- DRAM I/O (host-staged layouts) ----------------
    d_xslab = nc.dram_tensor("xslab", [2, P, SROWS * WP], BF16, kind="ExternalInput")
    d_xT = nc.dram_tensor("xT", [TR_ALLOC, 2 * C], BF16, kind="ExternalInput")
    d_offw = nc.dram_tensor("offw", [P, NBR * K * 2 * 18], BF16, kind="ExternalInput")
    d_convw = nc.dram_tensor("convw", [P, 4 * P], BF16, kind="ExternalInput")
    d_convb = nc.dram_tensor("convb", [1, 2 * P], BF16, kind="ExternalInput")
    d_defw = nc.dram_tensor("defw", [P, NBR * K * C], BF16, kind="ExternalInput")
    d_w1T = nc.dram_tensor("w1T", [P, 2 * 512], BF16, kind="ExternalInput")
    d_b1row = nc.dram_tensor("b1row", [1, 512], BF16, kind="ExternalInput")
    d_w2T = nc.dram_tensor("w2T", [P, 4 * C], BF16, kind="ExternalInput")
    d_b2row = nc.dram_tensor("b2row", [1, C], BF16, kind="ExternalInput")
    d_ybrel = nc.dram_tensor("ybrel", [P, NBR * RB * K], BF16, kind="ExternalInput")
    d_ybabs = nc.dram_tensor("ybabs", [P, NBR * RB * K], BF16, kind="ExternalInput")
    d_xvb = nc.dram_tensor("xvb", [P, NBR * RB * K], BF16, kind="ExternalInput")
    d_ident = nc.dram_tensor("identf", [P, P], F32, kind="ExternalInput")
    d_identb = nc.dram_tensor("identb", [P, P], BF16, kind="ExternalInput")
    d_ones1 = nc.dram_tensor("ones1", [1, P], BF16, kind="ExternalInput")
    d_out = nc.dram_tensor("out", [RB * P, C], F32, kind="ExternalOutput")

    with tile.TileContext(nc) as tc:
        import contextlib
        ctx = contextlib.ExitStack()
        with ctx:
            cpool = ctx.enter_context(tc.tile_pool(name="const", bufs=1))
            spool = ctx.enter_context(tc.tile_pool(name="slab", bufs=1))
            tpool = ctx.enter_context(tc.tile_pool(name="tables", bufs=1))
            wpool = ctx.enter_context(tc.tile_pool(name="work", bufs=1))
            gpool = ctx.enter_context(tc.tile_pool(name="gath", bufs=2))
            apool = ctx.enter_context(tc.tile_pool(name="apply", bufs=2))
            mpool = ctx.enter_context(tc.tile_pool(name="mlp", bufs=2))
            pspool = ctx.enter_context(tc.tile_pool(name="ps", bufs=2, space="PSUM"))

            # ---- constants ----
            def load_const(name, dram, shape, dtype):
                t = cpool.tile(shape, dtype, tag=name, name=name)
                nc.sync.dma_start(t[:], dram[:])
                return t

            ident = load_const("ident", d_ident, [P, P], F32)
            identb = load_const("identb", d_identb, [P, P], BF16)
            ones1 = load_const("ones1", d_ones1, [1, P], BF16)
            convb = load_const("convb", d_convb, [1, 2 * P], BF16)
            offw = load_const("offw", d_offw, [P, NBR * K * 2 * 18], BF16)
            convw = load_const("convw", d_convw, [P, 4 * P], BF16)
            defw = load_const("defw", d_defw, [P, NBR * K * C], BF16)
            w1T = load_const("w1T", d_w1T, [P, 2 * 512], BF16)
            w2T = load_const("w2T", d_w2T, [P, 4 * C], BF16)
            b1row = load_const("b1row", d_b1row, [1, 512], BF16)
            b2row = load_const("b2row", d_b2row, [1, C], BF16)
            ybrel = load_const("ybrel", d_ybrel, [P, NBR * RB * K], BF16)
            ybabs = load_const("ybabs", d_ybabs, [P, NBR * RB * K], BF16)
            xvb = load_const("xvb", d_xvb, [P, NBR * RB * K], BF16)

            xslab = [spool.tile([P, SROWS, WP], BF16, tag=f"xs{kt}", name=f"xs{kt}") for kt in range(2)]
            for kt in range(2):
                nc.sync.dma_start(
                    xslab[kt][:], d_xslab[kt].rearrange("p (r w) -> p r w", w=WP))

            def offw_ap(br, tap, kt):
                base = ((br * K + tap) * 2 + kt) * 18
                return offw[:, base:base + 18]

            # ---- stage 1+2: offset convs -> transpose -> offT [128, RB, 96] ----
            # (each branch's 18 offset channels at partition base 32*br: PE
            #  output base partitions must be 32-aligned)
            offT = tpool.tile([P, RB, 96], F32)
            nc.vector.memset(offT[:], 0.0)
            for chunk in range(RB // 4):          # 4 rows = 512 px per chunk
                ps_off = pspool.tile([96, 512], F32, tag="ps2k", bufs=2)
                for br in range(NBR):
                    dil = DILS[br]
                    for tap in range(K):
                        dy, dx = tap // 3, tap % 3
                        srow = 4 * chunk + HALO + (dy - 1) * dil
                        scol = HALO + (dx - 1) * dil
                        for kt in range(2):
                            nc.tensor.matmul(
                                ps_off[br * 32:br * 32 + 18, :],
                                lhsT=offw_ap(br, tap, kt),
                                rhs=xslab[kt][:, srow:srow + 4, scol:scol + P],
                                start=(tap == 0 and kt == 0),
                                stop=(tap == K - 1 and kt == 1),
                            )
                osb = wpool.tile([96, 512], F32, tag="osb")
                for br in range(NBR):
                    nc.any.tensor_copy(osb[br * 32:br * 32 + 18, :],
                                       ps_off[br * 32:br * 32 + 18, :])
                for sub in range(4):
                    r = chunk * 4 + sub
                    for br in range(NBR):
                        ps_t = pspool.tile([P, 18], F32, tag="ps_sm", bufs=1)
                        nc.tensor.transpose(
                            ps_t[:], osb[br * 32:br * 32 + 18, sub * P:(sub + 1) * P],
                            ident[br * 32:br * 32 + 18, br * 32:br * 32 + 18])
                        nc.any.tensor_copy(offT[:, r, br * 32:br * 32 + 18], ps_t[:])

            # ---- stage 3: per-branch tables ----
            s00 = tpool.tile([P, NBR, RB, K], F32)
            s01 = tpool.tile([P, NBR, RB, K], F32)
            s10 = tpool.tile([P, NBR, RB, K], F32)
            s11 = tpool.tile([P, NBR, RB, K], F32)
            idx0 = tpool.tile([P, NBR, RB, K], I32)

            shp = [P, RB, K]
            NE = RB * K

            def wt(tag):
                return wpool.tile(shp, F32, tag=tag, name=tag)

            def floor_chain(dst, src_ap):
                nc.vector.tensor_scalar(
                    out=dst[:], in0=src_ap, scalar1=-2.0, scalar2=-3.0,
                    op0=A_OP.is_ge, op1=A_OP.add)
                for t in (-1.0, 0.0, 1.0, 2.0):
                    cmp_t = wt("cmp")
                    nc.vector.tensor_scalar(
                        out=cmp_t[:], in0=src_ap, scalar1=t, scalar2=None,
                        op0=A_OP.is_ge)
                    nc.vector.tensor_tensor(
                        out=dst[:], in0=dst[:], in1=cmp_t[:], op=A_OP.add)

            def bound_mask(dst, src, lo, hi):
                m2 = wt("mtmp")
                nc.vector.tensor_scalar(out=dst[:], in0=src[:], scalar1=lo,
                                        scalar2=None, op0=A_OP.is_ge)
                nc.vector.tensor_scalar(out=m2[:], in0=src[:], scalar1=hi,
                                        scalar2=None, op0=A_OP.is_le)
                nc.vector.tensor_tensor(out=dst[:], in0=dst[:], in1=m2[:],
                                        op=A_OP.mult)

            part_dim = offT[:].ap[0]
            base_off = offT[:].offset
            for br in range(NBR):
                # strided views of offT: channel (br*18 + 2k) (+1 for x-offset)
                offy_v = AP(offT.tensor, base_off + br * 32,
                            [part_dim, [96, RB], [2, K]])
                offx_v = AP(offT.tensor, base_off + br * 32 + 1,
                            [part_dim, [96, RB], [2, K]])
                cst = lambda t: t[:, br * NE:(br + 1) * NE].rearrange(
                    "p (r k) -> p r k", k=K)
                ybrel_v = cst(ybrel)
                ybabs_v = cst(ybabs)
                xvb_v = cst(xvb)

                fy = wt("fy")
                floor_chain(fy, offy_v)
                fx = wt("fx")
                floor_chain(fx, offx_v)

                ay = wt("ay")
                nc.vector.tensor_tensor(out=ay[:], in0=offy_v, in1=fy[:],
                                        op=A_OP.subtract)
                ax = wt("ax")
                nc.vector.tensor_tensor(out=ax[:], in0=offx_v, in1=fx[:],
                                        op=A_OP.subtract)
                y0a = wt("y0a")
                nc.vector.tensor_tensor(out=y0a[:], in0=ybabs_v, in1=fy[:],
                                        op=A_OP.add)
                x0a = wt("x0a")
                nc.vector.tensor_tensor(out=x0a[:], in0=xvb_v, in1=fx[:],
                                        op=A_OP.add)

                msk = wt("msk")
                wy0 = wt("wy0")
                bound_mask(msk, y0a, 0.0, float(H - 1))
                nc.vector.tensor_scalar(out=wy0[:], in0=ay[:], scalar1=-1.0,
                                        scalar2=1.0, op0=A_OP.mult, op1=A_OP.add)
                nc.vector.tensor_tensor(out=wy0[:], in0=wy0[:], in1=msk[:],
                                        op=A_OP.mult)
                msk2 = wt("msk2")
                wy1 = wt("wy1")
                bound_mask(msk2, y0a, -1.0, float(H - 2))
                nc.vector.tensor_tensor(out=wy1[:], in0=ay[:], in1=msk2[:],
                                        op=A_OP.mult)
                msk3 = wt("msk3")
                wx0 = wt("wx0")
                bound_mask(msk3, x0a, 0.0, float(W - 1))
                nc.vector.tensor_scalar(out=wx0[:], in0=ax[:], scalar1=-1.0,
                                        scalar2=1.0, op0=A_OP.mult, op1=A_OP.add)
                nc.vector.tensor_tensor(out=wx0[:], in0=wx0[:], in1=msk3[:],
                                        op=A_OP.mult)
                msk4 = wt("msk4")
                wx1 = wt("wx1")
                bound_mask(msk4, x0a, -1.0, float(W - 2))
                nc.vector.tensor_tensor(out=wx1[:], in0=ax[:], in1=msk4[:],
                                        op=A_OP.mult)

                nc.vector.tensor_tensor(out=s00[:, br], in0=wy0[:], in1=wx0[:],
                                        op=A_OP.mult)
                nc.vector.tensor_tensor(out=s01[:, br], in0=wy0[:], in1=wx1[:],
                                        op=A_OP.mult)
                nc.vector.tensor_tensor(out=s10[:, br], in0=wy1[:], in1=wx0[:],
                                        op=A_OP.mult)
                nc.vector.tensor_tensor(out=s11[:, br], in0=wy1[:], in1=wx1[:],
                                        op=A_OP.mult)

                idxf = wt("idxf")
                nc.vector.tensor_tensor(out=idxf[:], in0=ybrel_v, in1=fy[:],
                                        op=A_OP.add)
                nc.vector.tensor_scalar(out=idxf[:], in0=idxf[:], scalar1=float(W),
                                        scalar2=None, op0=A_OP.mult)
                nc.vector.tensor_tensor(out=idxf[:], in0=idxf[:], in1=x0a[:],
                                        op=A_OP.add)
                nc.vector.tensor_copy(idx0[:, br], idxf[:])

            xT_view = d_xT[:]
            _qctr = [0]

            # ---- per-LN helper: mu/rstd from (sumx, ssq) [P, n] ----
            lnp = wpool

            def ln_murstd(sumx, ssq, n, ngrp, tag):
                mu = lnp.tile([P, ngrp], F32, tag=tag + "mu", name=tag + "mu",
                              padded_shape=[P, 4])
                nc.vector.tensor_scalar(out=mu[:], in0=sumx[:], scalar1=1.0 / n,
                                        scalar2=None, op0=A_OP.mult)
                mu2 = lnp.tile([P, ngrp], F32, tag=tag + "mu2", name=tag + "mu2",
                               padded_shape=[P, 4])
                nc.vector.tensor_tensor(out=mu2[:], in0=mu[:], in1=mu[:],
                                        op=A_OP.mult)
                var = lnp.tile([P, ngrp], F32, tag=tag + "var", name=tag + "var",
                               padded_shape=[P, 4])
                nc.vector.scalar_tensor_tensor(
                    out=var[:], in0=ssq[:], scalar=1.0 / n, in1=mu2[:],
                    op0=A_OP.mult, op1=A_OP.subtract)
                nc.vector.tensor_scalar(out=var[:], in0=var[:], scalar1=EPS,
                                        scalar2=None, op0=A_OP.add)
                # Newton rsqrt (2 iters from bit-trick seed; ACT Sqrt would
                # force an act-table swap away from the gelu set)
                vi = lnp.tile([P, ngrp], I32, tag=tag + "vi", name=tag + "vi",
                              padded_shape=[P, 4])
                nc.vector.tensor_scalar(
                    out=vi[:], in0=var[:].bitcast(I32), scalar1=1,
                    scalar2=None, op0=A_OP.arith_shift_right)
                nc.vector.tensor_scalar(
                    out=vi[:], in0=vi[:], scalar1=-1, scalar2=0x5f3759df,
                    op0=A_OP.mult, op1=A_OP.add)
                y = lnp.tile([P, ngrp], F32, tag=tag + "rs", name=tag + "rs",
                             padded_shape=[P, 4])
                nc.vector.tensor_copy(y[:], vi[:].bitcast(F32))
                t2 = lnp.tile([P, ngrp], F32, tag=tag + "t2", name=tag + "t2",
                              padded_shape=[P, 4])
                for _ in range(2):
                    nc.vector.tensor_tensor(out=t2[:], in0=y[:], in1=y[:],
                                            op=A_OP.mult)
                    nc.vector.tensor_tensor(out=t2[:], in0=t2[:], in1=var[:],
                                            op=A_OP.mult)
                    nc.vector.tensor_scalar(out=t2[:], in0=t2[:], scalar1=-0.5,
                                            scalar2=1.5, op0=A_OP.mult,
                                            op1=A_OP.add)
                    nc.vector.tensor_tensor(out=y[:], in0=y[:], in1=t2[:],
                                            op=A_OP.mult)
                return mu, y

            # ---- stage 4: per-row pipeline ----
            for r in range(RB):
                glu = apool.tile([P, NBR * C], BF16, tag="glu")
                sumx3 = wpool.tile([P, NBR], F32, tag="sumx3", name="sumx3",
                                   padded_shape=[P, 4])
                ssq3 = wpool.tile([P, NBR], F32, tag="ssq3", name="ssq3",
                                  padded_shape=[P, 4])
                ysums = []
                for br in range(NBR):
                    g0 = gpool.tile([P, K, 4 * C], BF16, tag="g", bufs=2)
                    for k in range(K):
                        gi = nc.gpsimd.indirect_dma_start(
                            out=g0[:, k, :], out_offset=None, in_=xT_view,
                            in_offset=bass.IndirectOffsetOnAxis(
                                ap=idx0[:, br, r, k:k + 1], axis=0))
                        qn = _qctr[0] % 4
                        gi.queue = f"qPoolDynamic{qn if qn else ''}"
                        _qctr[0] += 1

                    acc = apool.tile([P, K, C], BF16, tag="acc")
                    for k in range(K):
                        a_sl = acc[:, k, :]
                        tp = apool.tile([P, 2, C], BF16, tag="tp4", bufs=6)
                        # 4 tap products on DVE tensor_scalar (4x mode);
                        # adds on tensor_tensor (2x) — scalar_tensor_tensor
                        # only has a 1x uop, so the fused form is slower
                        nc.vector.tensor_scalar(
                            out=a_sl, in0=g0[:, k, 0:C],
                            scalar1=s00[:, br, r, k:k + 1], scalar2=None,
                            op0=A_OP.mult)
                        nc.vector.tensor_scalar(
                            out=tp[:, 0, :], in0=g0[:, k, C:2 * C],
                            scalar1=s10[:, br, r, k:k + 1], scalar2=None,
                            op0=A_OP.mult)
                        nc.vector.tensor_scalar(
                            out=tp[:, 1, :], in0=g0[:, k, 2 * C:3 * C],
                            scalar1=s01[:, br, r, k:k + 1], scalar2=None,
                            op0=A_OP.mult)
                        nc.any.tensor_tensor(out=a_sl, in0=a_sl,
                                             in1=tp[:, 0, :], op=A_OP.add)
                        nc.vector.tensor_scalar(
                            out=tp[:, 0, :], in0=g0[:, k, 3 * C:4 * C],
                            scalar1=s11[:, br, r, k:k + 1], scalar2=None,
                            op0=A_OP.mult)
                        nc.any.tensor_tensor(out=tp[:, 1, :], in0=tp[:, 1, :],
                                             in1=tp[:, 0, :], op=A_OP.add)
                        nc.any.tensor_tensor(out=a_sl, in0=a_sl,
                                             in1=tp[:, 1, :], op=A_OP.add)
                    # depthwise weights: one big multiply over all 9 taps
                    dw_eng = nc.gpsimd if br < GPS_DEFW else nc.vector
                    dw_eng.tensor_tensor(
                        out=acc[:].rearrange("p a b -> p (a b)"),
                        in0=acc[:].rearrange("p a b -> p (a b)"),
                        in1=defw[:, br * K * C:(br + 1) * K * C],
                        op=A_OP.mult)
                    # k-sum on PE: 9 identity-matmuls accumulate into PSUM
                    ps_df = pspool.tile([P, C], F32, tag="psdf", bufs=2)
                    for k in range(K):
                        nc.tensor.matmul(ps_df[:], lhsT=identb[:],
                                         rhs=acc[:, k, :],
                                         start=(k == 0), stop=(k == K - 1))
                    ysum = apool.tile([P, C], BF16, tag=f"ysum{br}",
                                      name=f"ysum{br}")
                    nc.scalar.activation(out=ysum[:], in_=ps_df[:], func=AF.Copy,
                                         accum_out=sumx3[:, br:br + 1])
                    sq = apool.tile([P, C], BF16, tag="sqscr", bufs=4)
                    nc.scalar.activation(out=sq[:], in_=ysum[:], func=AF.Square,
                                         accum_out=ssq3[:, br:br + 1])
                    ysums.append(ysum)

                mu3, rstd3 = ln_murstd(sumx3, ssq3, C, NBR, "l3")
                nmr3 = lnp.tile([P, NBR], F32, tag="nmr3", name="nmr3",
                                padded_shape=[P, 4])
                nc.vector.scalar_tensor_tensor(
                    out=nmr3[:], in0=mu3[:], scalar=-1.0, in1=rstd3[:],
                    op0=A_OP.mult, op1=A_OP.mult)
                for br in range(NBR):
                    nc.scalar.activation(
                        out=glu[:, br * C:(br + 1) * C], in_=ysums[br][:],
                        func=AF.Gelu, scale=rstd3[:, br:br + 1],
                        bias=nmr3[:, br:br + 1])

                # 1x1 conv for this row (bias preloaded into PSUM)
                c1 = mpool.tile([P, C], BF16, tag="c1")
                for ct in range(2):
                    ps_c = pspool.tile([P, P], F32, tag="ps_sm", bufs=1)
                    nc.tensor.matmul(
                        ps_c[:], lhsT=convb[:, ct * P:(ct + 1) * P],
                        rhs=ones1[:], start=True, stop=False)
                    for kt in range(2):
                        nc.tensor.matmul(
                            ps_c[:],
                            lhsT=convw[:, (kt * 2 + ct) * P:(kt * 2 + ct + 1) * P],
                            rhs=xslab[kt][:, HALO + r, HALO:HALO + P],
                            start=False, stop=(kt == 1))
                    cb = mpool.tile([P, P], BF16, tag="cb")
                    nc.scalar.activation(out=cb[:], in_=ps_c[:], func=AF.Copy)
                    ps_ct = pspool.tile([P, P], BF16, tag="ps_smb")
                    nc.tensor.transpose(ps_ct[:], cb[:], identb[:])
                    nc.scalar.activation(out=c1[:, ct * P:(ct + 1) * P],
                                         in_=ps_ct[:], func=AF.Copy)

                # branch sum + 1x1 conv + LN
                sum1 = wpool.tile([P, 1], F32, tag="sum1", name="sum1",
                                  padded_shape=[P, 4])
                ssq1 = wpool.tile([P, 1], F32, tag="ssq1", name="ssq1",
                                  padded_shape=[P, 4])
                tot = mpool.tile([P, C], BF16, tag="tot")
                nc.vector.tensor_tensor(out=tot[:], in0=glu[:, 0:C],
                                        in1=glu[:, C:2 * C], op=A_OP.add)
                nc.vector.tensor_tensor(out=tot[:], in0=tot[:],
                                        in1=glu[:, 2 * C:3 * C], op=A_OP.add)
                nc.vector.scalar_tensor_tensor(
                    out=tot[:], in0=c1[:], scalar=1.0, in1=tot[:],
                    op0=A_OP.mult, op1=A_OP.add, accum_out=sum1[:])
                sq1 = apool.tile([P, C], BF16, tag="sqscr", bufs=4)
                nc.scalar.activation(out=sq1[:], in_=tot[:], func=AF.Square,
                                     accum_out=ssq1[:])
                mu1, rstd1 = ln_murstd(sum1, ssq1, C, 1, "l1")
                outr = mpool.tile([P, C], BF16, tag="outr")
                nc.vector.tensor_scalar(
                    out=outr[:], in0=tot[:], scalar1=mu1[:, 0:1],
                    scalar2=rstd1[:, 0:1], op0=A_OP.subtract, op1=A_OP.mult)

                # MLP
                outT = mpool.tile([P, 2, P], BF16, tag="outT")
                for ct in range(2):
                    ps_tr = pspool.tile([P, P], BF16, tag="ps_smb")
                    nc.tensor.transpose(ps_tr[:], outr[:, ct * P:(ct + 1) * P],
                                        identb[:])
                    nc.scalar.activation(out=outT[:, ct, :], in_=ps_tr[:],
                                         func=AF.Copy)

                ps_h = pspool.tile([P, 512], F32, tag="ps2k")
                for ct in range(2):
                    nc.tensor.matmul(ps_h[:], lhsT=outT[:, ct, :],
                                     rhs=w1T[:, ct * 512:(ct + 1) * 512],
                                     start=(ct == 0), stop=False)
                nc.tensor.matmul(ps_h[:], lhsT=ones1[:], rhs=b1row[:],
                                 start=False, stop=True)
                hg = mpool.tile([P, 512], BF16, tag="hg")
                nc.scalar.activation(out=hg[:], in_=ps_h[:], func=AF.Gelu)

                hT = mpool.tile([P, 4, P], BF16, tag="hT")
                for jt in range(4):
                    ps_ht = pspool.tile([P, P], BF16, tag="ps_smb")
                    nc.tensor.transpose(ps_ht[:], hg[:, jt * P:(jt + 1) * P],
                                        identb[:])
                    nc.scalar.activation(out=hT[:, jt, :], in_=ps_ht[:],
                                         func=AF.Copy)

                ps_o = pspool.tile([P, C], F32, tag="pso", bufs=1)
                for jt in range(4):
                    nc.tensor.matmul(ps_o[:], lhsT=hT[:, jt, :],
                                     rhs=w2T[:, jt * C:(jt + 1) * C],
                                     start=(jt == 0), stop=False)
                nc.tensor.matmul(ps_o[:], lhsT=ones1[:], rhs=b2row[:],
                                 start=False, stop=True)

                # residual + LN
                sum2 = wpool.tile([P, 1], F32, tag="sum2", name="sum2",
                                  padded_shape=[P, 4])
                ssq2 = wpool.tile([P, 1], F32, tag="ssq2", name="ssq2",
                                  padded_shape=[P, 4])
                res = mpool.tile([P, C], F32, tag="res")
                nc.vector.scalar_tensor_tensor(
                    out=res[:], in0=ps_o[:], scalar=1.0, in1=outr[:],
                    op0=A_OP.mult, op1=A_OP.add, accum_out=sum2[:])
                sq2 = apool.tile([P, C], BF16, tag="sqscr", bufs=4)
                nc.scalar.activation(out=sq2[:], in_=res[:], func=AF.Square,
                                     accum_out=ssq2[:])
                mu2r, rstd2 = ln_murstd(sum2, ssq2, C, 1, "l2")
                orow = mpool.tile([P, C], F32, tag="orow")
                nc.vector.tensor_scalar(
                    out=orow[:], in0=res[:], scalar1=mu2r[:, 0:1],
                    scalar2=rstd2[:, 0:1], op0=A_OP.subtract, op1=A_OP.mult)
                nc.sync.dma_start(d_out[r * P:(r + 1) * P, :], orow[:])

    nc.compile()
    return nc


def stage_inputs(inputs):
    """Build per-core in_maps from full inputs (layout/dtype staging only)."""
    x = np.asarray(inputs["x"], np.float32)
    off_w = [np.asarray(inputs[f"off_w{i}"], np.float32) for i in (1, 2, 3)]
    def_w = [np.asarray(inputs[f"def_w{i}"], np.float32) for i in (1, 2, 3)]
    conv_w = np.asarray(inputs["conv_w"], np.float32)[:, :, 0, 0]
    conv_b = np.asarray(inputs["conv_b"], np.float32)
    w1 = np.asarray(inputs["mlp_w1"], np.float32)
    b1 = np.asarray(inputs["mlp_b1"], np.float32)
    w2 = np.asarray(inputs["mlp_w2"], np.float32)
    b2 = np.asarray(inputs["mlp_b2"], np.float32)

    bf = lambda a: np.ascontiguousarray(a, np.float32).astype(mybir.dt.np(BF16))

    offw = np.zeros((NBR, K, 2, P, 18), np.float32)
    for br in range(NBR):
        for tap in range(K):
            ky, kx = tap // 3, tap % 3
            for kt in range(2):
                offw[br, tap, kt] = off_w[br][:, kt * P:(kt + 1) * P, ky, kx].T
    offw_f = offw.transpose(3, 0, 1, 2, 4).reshape(P, NBR * K * 2 * 18)

    convw = np.zeros((2, 2, P, P), np.float32)
    for kt in range(2):
        for ct in range(2):
            convw[kt, ct] = conv_w[ct * P:(ct + 1) * P, kt * P:(kt + 1) * P].T
    convw_f = convw.transpose(2, 0, 1, 3).reshape(P, 4 * P)

    convb_f = conv_b[None, :].copy()     # [1, 256]

    defw = np.zeros((NBR, K, P, C), np.float32)
    for br in range(NBR):
        dwk = def_w[br][:, 0].reshape(C, K)
        for k in range(K):
            defw[br, k] = np.broadcast_to(dwk[:, k][None, :], (P, C))
    defw_f = defw.transpose(2, 0, 1, 3).reshape(P, NBR * K * C)

    w1T_f = np.concatenate([w1.T[:P], w1.T[P:]], axis=1)        # [128, 2*512]
    w2T_f = np.concatenate([w2.T[jt * P:(jt + 1) * P] for jt in range(4)], axis=1)

    identf = np.eye(P, dtype=np.float32)
    ones1 = np.ones((1, P), np.float32)

    shared = dict(
        offw=bf(offw_f), convw=bf(convw_f), convb=bf(convb_f), defw=bf(defw_f),
        w1T=bf(w1T_f), b1row=bf(b1[None, :]), w2T=bf(w2T_f), b2row=bf(b2[None, :]),
        identf=identf, identb=bf(identf), ones1=bf(ones1),
    )

    in_maps = []
    xr = x.reshape(B, 2, P, H, W)
    for core in range(NCORES):
        b = core // 4
        y0 = (core % 4) * RB
        slab = np.zeros((2, P, SROWS, WP), np.float32)
        rlo, rhi = y0 - HALO, y0 + RB + HALO
        srlo, srhi = max(rlo, 0), min(rhi, H)
        slab[:, :, srlo - rlo:srhi - rlo, HALO:HALO + W] = xr[b][:, :, srlo:srhi]
        xT1 = np.zeros((TR_ALLOC + W, C), np.float32)
        tlo = y0 - TPAD
        alo, ahi = max(tlo, 0), min(y0 + RB + TPAD + 1, H)
        xT1[(alo - tlo) * W:(ahi - tlo) * W] = \
            x[b, :, alo:ahi, :].reshape(C, -1).T
        xT = np.concatenate([xT1[:TR_ALLOC], xT1[W:TR_ALLOC + W]], axis=1)
        ybrel = np.zeros((P, NBR, RB, K), np.float32)
        ybabs = np.zeros((P, NBR, RB, K), np.float32)
        xvb = np.zeros((P, NBR, RB, K), np.float32)
        for br in range(NBR):
            dil = DILS[br]
            for k in range(K):
                ky, kx = k // 3, k % 3
                rows = y0 + np.arange(RB) + (ky - 1) * dil
                ybabs[:, br, :, k] = rows[None, :]
                ybrel[:, br, :, k] = rows[None, :] - y0 + TPAD
                xvb[:, br, :, k] = (np.arange(P) + (kx - 1) * dil)[:, None]
        m = dict(shared)
        m.update(xslab=bf(slab.reshape(2, P, SROWS * WP)), xT=bf(xT),
                 ybrel=bf(ybrel.reshape(P, -1)), ybabs=bf(ybabs.reshape(P, -1)),
                 xvb=bf(xvb.reshape(P, -1)))
        in_maps.append(m)
    return in_maps


def assemble_output(results):
    out = np.zeros((B, C, H, W), np.float32)
    for core in range(NCORES):
        b = core // 4
        y0 = (core % 4) * RB
        o = np.asarray(results[core]["out"], np.float32)
        out[b, :, y0:y0 + RB, :] = o.reshape(RB, W, C).transpose(2, 0, 1)
    return out


def kernel(**inputs):
    global _COMPILED
    from concourse.bass_utils import run_bass_kernel_spmd
    if _COMPILED is None:
        _COMPILED = build_program()
    nc = _COMPILED
    in_maps = stage_inputs(inputs)
    res = run_bass_kernel_spmd(nc, in_maps, core_ids=list(range(NCORES)))
    return assemble_output(res.results)
